# revision 21
# baseline (speedup 1.0000x reference)
"""Distributed sliding-window GQA attention kernel for 8 Trainium2 NeuronCores.

Problem (full shapes): x [1, 2048, 4096] f32, wq [4096, 4096], wk/wv [4096, 1024],
wo [4096, 4096], rotary freqs [2048, 64]. 32 q heads / 8 kv heads (GQA group 4),
head_dim 128, causal sliding window 1024.

Sharding (tensor parallel over heads): core c owns q heads 4c..4c+3 and kv head c
(wq/wk/wv column shards). The output projection is column-sharded: each core
AllGathers the (bf16) attention outputs per 512-token chunk and computes
out[:, 512c:512c+512] with its wo column shard. Host concatenates.

v3 schedule notes (on top of v2's fused-wqkv / resident-wo layout):
 - emit_qkv computes k and v BEFORE the q heads so per-head attention can
   start as soon as its own q projection lands (dataflow overlap with the
   remaining q-head matmuls).
 - softmax denominators come from a DVE-accumulated esum ([128,512] f32,
   cast to bf16) + ONE ones-matmul per head, instead of a PE row-sum
   matmul per j-block (saves ~60K PE cycles/chunk).
 - masks are two [128,128] triangles applied only to the diagonal 128-col
   sub-block of each masked j-block (the rest of the span is fully valid).
 - gather readbacks are issued from the (otherwise idle) sync engine so a
   late AllGather cannot head-of-line block gpsimd's partition_broadcasts
   and ccin ships.
 - outproj contracts gather-part-0 rows before part-1 rows so PE has ready
   work while part 1 is still in flight; chunk 3 uses the same 2-part
   gathers as other chunks but a 4-bank single pass in part-major order.
 - a tiny warmup AllGather issues at t~0 to absorb the first-collective
   rendezvous/barrier cost (~50us) during the DMA-bound prologue.
"""

import math
from contextlib import ExitStack

import numpy as np

import concourse.bass as bass
import concourse.mybir as mybir
import concourse.tile as tile
from concourse import bacc
from concourse.bass_utils import run_bass_kernel_spmd
from concourse.masks import make_identity

# ---- problem constants (hardcoded; kernel.py must be self-contained) ----
B = 1
S = 2048
D = 4096
N_Q_HEADS = 32
HD = 128
WINDOW = 1024
N_CORES = 8

QH = N_Q_HEADS // N_CORES  # 4 local q heads
P = 128
CH = 512  # seq chunk
NCH = S // CH  # 4
DT = D // P  # 32 contraction tiles
ST = CH // P  # 4 s-tiles per chunk
DC = D // N_CORES  # 512 output columns per core
QKV = QH * HD + 2 * HD  # 768 fused qkv columns
HPG = 2  # heads per gather part
NAG = QH // HPG  # gather parts per chunk (2)

F32 = mybir.dt.float32
BF16 = mybir.dt.bfloat16

_BUILT = None


def _span(rel):
    """Non-zero column span (c0, c1) and mask kind for a j-block at relative
    position rel = (j0 - (i0 - WINDOW)) // 128 in 0..11.

    mask kind: None = fully valid; 'w' = window-edge triangle on cols
    [c1-128, c1); 'c' = causal triangle on cols [c0, c0+128)."""
    if rel <= 3:
        return 0, 128 * (rel + 1), 'w'  # window-edge wedge
    if rel <= 7:
        return 0, CH, None  # fully inside window
    return 128 * (rel - 8), CH, 'c'  # causal wedge


def _build():
    nc = bacc.Bacc("TRN2", target_bir_lowering=False, debug=False, num_devices=N_CORES)

    xT_ext = nc.declare_dram_parameter("xT", [D, S], BF16, isOutput=False)
    wqkv_ext = nc.declare_dram_parameter("wqkv", [D, QKV], BF16, isOutput=False)
    wo_ext = nc.declare_dram_parameter("wo", [D, DC], BF16, isOutput=False)
    cossin_ext = nc.declare_dram_parameter("cossinT", [P, 2, S], BF16, isOutput=False)
    mask_ext = nc.declare_dram_parameter("masks", [2, P, P], BF16, isOutput=False)
    out_ext = nc.declare_dram_parameter("out", [S, DC], BF16, isOutput=True)

    inv_sqrt_hd = 1.0 / math.sqrt(HD)

    with tile.TileContext(nc) as tc:
        with ExitStack() as stack:
            pool = lambda *a, **kw: stack.enter_context(tc.tile_pool(*a, **kw))
            wqkv_pool = pool(name="wqkv", bufs=4)  # 4 x [128, 8, 768]
            wo_pool = pool(name="wo", bufs=1)  # [128, 32, 512] resident
            x_pool = pool(name="xbf", bufs=5)  # [128, 8, 512] tiles
            k_pool = pool(name="kt", bufs=NCH)
            v_pool = pool(name="vt", bufs=12)
            q_pool = pool(name="qt", bufs=4)
            att_pool = pool(name="att", bufs=3)
            e_pool = pool(name="et", bufs=6)
            es_pool = pool(name="esum", bufs=2)
            r_pool = pool(name="rtmp", bufs=2)
            vts_pool = pool(name="vts", bufs=1)
            rb_pool = pool(name="rb", bufs=2)
            rc_pool = pool(name="rc", bufs=2)
            at_pool = pool(name="atst", bufs=4)  # [128, 8, 512] gathered att
            out_pool = pool(name="osb", bufs=1)  # [128, 4, 512] f32 staging
            misc_pool = pool(name="misc", bufs=1)
            mask_pool = pool(name="mask", bufs=1)
            acc_ps = pool(name="accps", bufs=2, space="PSUM")
            qkv_ps = acc_ps
            s_ps = pool(name="spsps", bufs=2, space="PSUM")  # sps + rs
            pv_ps = pool(name="pvps", bufs=2, space="PSUM")
            rs_ps = s_ps
            op_ps = pool(name="opps", bufs=2, space="PSUM")
            ccin_pool = pool(name="ccin", bufs=10, space="DRAM")
            gath_pool = pool(name="gath", bufs=10, space="DRAM")

            # ---- small constants (needed by chunk 0) ----
            cossin_sb = misc_pool.tile([P, 2, S], BF16, tag="cossin")
            ident = misc_pool.tile([P, P], BF16, tag="ident")
            make_identity(nc, ident[:])
            ones_bf = misc_pool.tile([P, 1], BF16, tag="ones")
            nc.vector.memset(ones_bf[:], 1.0)
            mask_all = mask_pool.tile([P, 2, P], BF16, tag="mask", name="mask_all")
            mask_w = mask_all[:, 0, :]  # valid iff col <= row
            mask_c = mask_all[:, 1, :]  # valid iff col >= row

            # tile handles
            wqkv_t = [None] * 4  # [128, 8, 768] bf16
            wo_sb = [None]  # [128, 32, 512] bf16
            x_tiles = {}  # (I, g) -> [128, 8, 512] bf16
            k_chunks = [None] * NCH
            v_tiles = [None] * (NCH * ST)
            q_tiles = {}
            att_tiles = {}
            ccin = {}  # (I, part) -> [HPG*HD, CH] DRAM
            gath = {}  # (I, part) -> [HPG*HD*N_CORES, CH] DRAM shared

            def wq_ap(Dt, h):
                return wqkv_t[Dt // 8][:, Dt % 8, h * HD : (h + 1) * HD]

            def wk_ap(Dt):
                return wqkv_t[Dt // 8][:, Dt % 8, QH * HD : QH * HD + HD]

            def wv_ap(Dt):
                return wqkv_t[Dt // 8][:, Dt % 8, QH * HD + HD : QKV]

            def x_ap(I, Dt):
                return x_tiles[(I, Dt // 8)][:, Dt % 8, :]

            def emit_x_group(I, g, eng=None):
                """Load x_T rows [1024g, 1024(g+1)) cols of chunk I (bf16)."""
                xb = x_pool.tile([P, 8, CH], BF16, tag="xbf", name=f"xbf{I}_{g}")
                if eng is None:
                    eng = nc.sync if g % 2 == 0 else nc.scalar
                eng.dma_start(
                    out=xb[:],
                    in_=xT_ext[
                        g * 8 * P : (g + 1) * 8 * P, I * CH : (I + 1) * CH
                    ].rearrange("(po pi) s -> pi po s", pi=P),
                )
                x_tiles[(I, g)] = xb

            def emit_x_chunk(I):
                for g in range(4):
                    # chunk 1 loads ride the lightest prologue ring (scalar)
                    # so they cannot steal bandwidth from the chunk-0 loads
                    # still draining on sync/gpsimd.
                    emit_x_group(I, g, eng=nc.scalar if I == 1 else None)

            def rope(ps, out_bf, I):
                # packed: cossin plane 0 = [cos;cos], plane 1 = [sin;sin], so
                # t1 = [pr*cos; pi*cos], t2 = [pr*sin; pi*sin] in 2 full-width
                # DVE ops; out_r = t1.hi - t2.lo, out_i = t2.hi + t1.lo.
                cc = cossin_sb[:, 0, I * CH : (I + 1) * CH]
                ss = cossin_sb[:, 1, I * CH : (I + 1) * CH]
                t1 = r_pool.tile([P, CH], F32, tag="m", name="t1")
                nc.vector.tensor_mul(t1[:], ps[:, :], cc)
                # cross products written at the base partition where they are
                # consumed (TensorTensor needs equal SB base partitions)
                t2 = r_pool.tile([P, CH], F32, tag="m", name="t2")
                nc.vector.tensor_mul(t2[0:64, :], ps[64:128, :], ss[64:128, :])
                nc.vector.tensor_mul(t2[64:128, :], ps[0:64, :], ss[0:64, :])
                nc.vector.tensor_sub(out_bf[0:64, :], t1[0:64, :], t2[0:64, :])
                nc.vector.tensor_add(out_bf[64:128, :], t1[64:128, :], t2[64:128, :])

            def emit_qkv_finish(I, psk, psv, psq):
                """rope k, transpose v, rope q heads from finished PSUM accs.
                PSUM->SBUF copies run on the scalar engine to keep DVE free
                for the attention chain."""
                kb = k_pool.tile([P, CH], BF16, tag="kb", name=f"kb{I}")
                rope(psk, kb, I)
                k_chunks[I] = kb
                vT = vts_pool.tile([P, CH], BF16, tag="vT", name=f"vT{I}")
                nc.scalar.copy(vT[:], psv[:])
                for sb in range(ST):
                    trp = qkv_ps.tile([P, P], BF16, tag="acc", name=f"trp{I}_{sb}")
                    nc.tensor.transpose(trp[:], vT[:, sb * P : (sb + 1) * P], ident[:])
                    vb = v_pool.tile([P, P], BF16, tag="vb", name=f"vb{I}_{sb}")
                    nc.scalar.copy(vb[:], trp[:])
                    v_tiles[I * ST + sb] = vb
                for h in range(QH):
                    qb = q_pool.tile([P, CH], BF16, tag="qb", name=f"qb{I}_{h}")
                    rope(psq[h], qb, I)
                    q_tiles[(I, h)] = qb

            def emit_qkv0():
                """Chunk 0 in two Dt-major passes (PE consumes each x/w group
                as it arrives instead of stalling on the full prologue load):
                pass A = {k, v, q0} so attention head 0 + the first ship can
                start ~40us earlier; pass B = {q1, q2, q3} back-to-back.
                Accumulators borrow the idle op/pv banks."""
                psk = qkv_ps.tile([P, CH], F32, tag="acc", name="psk0")
                psv = qkv_ps.tile([P, CH], F32, tag="acc", name="psv0")
                psq = [None] * QH
                psq[0] = op_ps.tile([P, CH], F32, tag="op", name="psq0_0")
                for g in range(4):
                    for o, acc, wf in (
                        (0, psk, wk_ap),
                        (1, psv, wv_ap),
                        (2, psq[0], lambda Dt: wq_ap(Dt, 0)),
                    ):
                        for Dt in range(g * 8, (g + 1) * 8):
                            nc.tensor.matmul(
                                acc[:],
                                wf(Dt),
                                x_ap(0, Dt),
                                start=(Dt == 0),
                                stop=(Dt == DT - 1),
                            )
                # k, v, q0 ready: rope/transpose them now (emit_qkv_finish
                # handles all heads; q1-3 ropes just land after pass B)
                psq[1] = op_ps.tile([P, CH], F32, tag="op", name="psq0_1")
                psq[2] = pv_ps.tile([P, CH], F32, tag="pv", name="psq0_2")
                psq[3] = pv_ps.tile([P, CH], F32, tag="pv", name="psq0_3")
                for g in range(4):
                    for h in (1, 2, 3):
                        for Dt in range(g * 8, (g + 1) * 8):
                            nc.tensor.matmul(
                                psq[h][:],
                                wq_ap(Dt, h),
                                x_ap(0, Dt),
                                start=(Dt == 0),
                                stop=(Dt == DT - 1),
                            )
                emit_qkv_finish(0, psk, psv, psq)

            def emit_qkv(I):
                # k and v FIRST so attention on head h only waits for its own
                # q projection.
                psk = qkv_ps.tile([P, CH], F32, tag="acc", name=f"psk{I}")
                for Dt in range(DT):
                    nc.tensor.matmul(
                        psk[:],
                        wk_ap(Dt),
                        x_ap(I, Dt),
                        start=(Dt == 0),
                        stop=(Dt == DT - 1),
                    )
                psv = qkv_ps.tile([P, CH], F32, tag="acc", name=f"psv{I}")
                for Dt in range(DT):
                    nc.tensor.matmul(
                        psv[:],
                        wv_ap(Dt),
                        x_ap(I, Dt),
                        start=(Dt == 0),
                        stop=(Dt == DT - 1),
                    )
                kb = k_pool.tile([P, CH], BF16, tag="kb", name=f"kb{I}")
                rope(psk, kb, I)
                k_chunks[I] = kb
                vT = vts_pool.tile([P, CH], BF16, tag="vT", name=f"vT{I}")
                nc.scalar.copy(vT[:], psv[:])
                for sb in range(ST):
                    trp = qkv_ps.tile([P, P], BF16, tag="acc", name=f"trp{I}_{sb}")
                    nc.tensor.transpose(trp[:], vT[:, sb * P : (sb + 1) * P], ident[:])
                    vb = v_pool.tile([P, P], BF16, tag="vb", name=f"vb{I}_{sb}")
                    nc.scalar.copy(vb[:], trp[:])
                    v_tiles[I * ST + sb] = vb
                for h in range(QH):
                    ps = qkv_ps.tile([P, CH], F32, tag="acc", name=f"psq{I}_{h}")
                    for Dt in range(DT):
                        nc.tensor.matmul(
                            ps[:],
                            wq_ap(Dt, h),
                            x_ap(I, Dt),
                            start=(Dt == 0),
                            stop=(Dt == DT - 1),
                        )
                    qb = q_pool.tile([P, CH], BF16, tag="qb", name=f"qb{I}_{h}")
                    rope(ps, qb, I)
                    q_tiles[(I, h)] = qb

            def emit_attn(I):
                for part in range(NAG):
                    ccin[(I, part)] = ccin_pool.tile(
                        [HPG * HD, CH], BF16, tag="ci", name=f"cin{I}_{part}"
                    )
                i0 = I * CH
                jlo = max(0, i0 - WINDOW)
                n_j = (i0 + CH - jlo) // P
                pending = [None]

                def flush_norm():
                    if pending[0] is not None:
                        pending[0]()
                        pending[0] = None

                # widest block first: its full-span et COPY-initializes esum
                # (no memset), later blocks accumulate their sub-spans.
                order = sorted(range(n_j), key=lambda idx: (
                    _span((jlo + idx * P - (i0 - WINDOW)) // P)[0]
                    - _span((jlo + idx * P - (i0 - WINDOW)) // P)[1],
                ))
                for h in range(QH):
                    pv = pv_ps.tile([P, CH], F32, tag="pv", name=f"pv{I}_{h}")
                    qb = q_tiles[(I, h)]
                    esum = es_pool.tile([P, CH], F32, tag="es", name=f"es{I}_{h}")
                    esb = es_pool.tile([P, CH], BF16, tag="esb", name=f"esb{I}_{h}")
                    for oi, idx in enumerate(order):
                        j0 = jlo + idx * P
                        rel = (j0 - (i0 - WINDOW)) // P
                        c0, c1, mk = _span(rel)
                        kb = k_chunks[j0 // CH]
                        koff = j0 % CH
                        sps = s_ps.tile([P, CH], F32, tag="acc", name=f"sps{I}_{h}_{idx}")
                        nc.tensor.matmul(
                            sps[:, c0:c1],
                            kb[:, koff : koff + P],
                            qb[:, c0:c1],
                            start=True,
                            stop=True,
                            skip_group_check=True,
                        )
                        et = e_pool.tile([P, CH], BF16, tag="et", name=f"et{I}_{h}_{idx}")
                        nc.scalar.activation(
                            et[:, c0:c1],
                            sps[:, c0:c1],
                            mybir.ActivationFunctionType.Exp,
                            scale=inv_sqrt_hd,
                        )
                        if mk == 'w':
                            nc.vector.tensor_mul(
                                et[:, c1 - P : c1], et[:, c1 - P : c1], mask_w
                            )
                        elif mk == 'c':
                            nc.vector.tensor_mul(
                                et[:, c0 : c0 + P], et[:, c0 : c0 + P], mask_c
                            )
                        if oi == 0:
                            nc.vector.tensor_copy(esum[:], et[:])
                        else:
                            nc.vector.tensor_add(
                                esum[:, c0:c1], esum[:, c0:c1], et[:, c0:c1]
                            )
                        nc.tensor.matmul(
                            pv[:, c0:c1],
                            v_tiles[j0 // P][:],
                            et[:, c0:c1],
                            start=(oi == 0),
                            stop=(oi == n_j - 1),
                            skip_group_check=True,
                        )
                    nc.scalar.copy(esb[:], esum[:])
                    # rs shares the sps bank family ([1,CH] output in row 0)
                    rsf = rs_ps.tile([P, CH], F32, tag="acc", name=f"rs{I}_{h}")
                    rs = rsf[0:1, :]
                    nc.tensor.matmul(
                        rs,
                        ones_bf[:],
                        esb[:],
                        start=True,
                        stop=True,
                        skip_group_check=True,
                    )
                    # Inline: DVE frees the rs bank, computes 1/rs, casts to
                    # bf16; gpsimd broadcasts it across partitions. The final
                    # scale + ship are deferred one head so the DVE multiply
                    # never blocks waiting on the gpsimd broadcast.
                    rss = rc_pool.tile([1, CH], F32, tag="rss", name=f"rss{I}_{h}")
                    nc.vector.tensor_copy(rss[:], rs)
                    rc = rc_pool.tile([1, CH], F32, tag="rc", name=f"rc{I}_{h}")
                    nc.vector.reciprocal_approx_fast(rc[:], rss[:])
                    rcb = rc_pool.tile([1, CH], BF16, tag="rcb", name=f"rcb{I}_{h}")
                    nc.vector.tensor_copy(rcb[:], rc[:])
                    rb = rb_pool.tile([P, CH], BF16, tag="rb", name=f"rb{I}_{h}")
                    nc.gpsimd.partition_broadcast(rb[:], rcb[:])
                    flush_norm()

                    def mk_norm(h=h, pv=pv, rb=rb):
                        def go():
                            ab = att_pool.tile(
                                [P, CH], BF16, tag="ab", name=f"ab{I}_{h}"
                            )
                            nc.vector.tensor_mul(ab[:], pv[:], rb[:])
                            att_tiles[(I, h)] = ab
                            nc.gpsimd.dma_start(
                                out=ccin[(I, h // HPG)][
                                    (h % HPG) * HD : (h % HPG + 1) * HD, :
                                ],
                                in_=ab[:],
                            )
                            if h % HPG == HPG - 1:
                                emit_ag(I, h // HPG)

                        return go

                    pending[0] = mk_norm()
                flush_norm()

            def emit_ag(I, part):
                go = gath_pool.tile(
                    [HPG * HD * N_CORES, CH],
                    BF16,
                    addr_space="Shared",
                    tag="go",
                    name=f"go{I}_{part}",
                )
                nc.gpsimd.collective_compute(
                    "AllGather",
                    mybir.AluOpType.bypass,
                    replica_groups=[list(range(N_CORES))],
                    ins=[ccin[(I, part)][:].opt()],
                    outs=[go[:].opt()],
                )
                gath[(I, part)] = go

            def emit_outproj(I):
                ob = out_pool.tile([P, ST, CH], BF16, tag="ob", name=f"ob{I}")
                # readback: 2 DMAs of [128, 8, 512] per gather part, issued
                # from the sync engine (keeps slow gathers off gpsimd's queue)
                ats = {}
                for part in range(NAG):
                    for t in range(2):
                        at = at_pool.tile(
                            [P, 8, CH], BF16, tag="at", name=f"at{I}_{part}_{t}"
                        )
                        nc.sync.dma_start(
                            out=at[:],
                            in_=gath[(I, part)][
                                t * 8 * P : (t + 1) * 8 * P, :
                            ].rearrange("(po pi) s -> pi po s", pi=P),
                        )
                        ats[(part, t)] = at

                def mm_rows(part):
                    # gathered row-group j of part = rank j//2, head-slot
                    # 2*part + j%2 -> global head index 4*(j//2) + 2*part + j%2
                    for j in range(16):
                        g = 4 * (j // 2) + HPG * part + (j % 2)
                        yield ats[(part, j // 8)], j % 8, g

                if I < NCH - 1:
                    # 2 PSUM banks: seq-tile pairs; within each pair contract
                    # part0's 16 rows first (ready earlier), then part1's.
                    for pk in range(2):
                        pso = [
                            op_ps.tile([P, CH], F32, tag="op", name=f"pso{I}_{pk}_{k}")
                            for k in range(2)
                        ]
                        mm = 0
                        for part in range(NAG):
                            for at, jj, g in mm_rows(part):
                                for k in range(2):
                                    st = pk * 2 + k
                                    nc.tensor.matmul(
                                        pso[k][:],
                                        at[:, jj, st * P : (st + 1) * P],
                                        wo_sb[0][:, g, :],
                                        start=(mm == 0),
                                        stop=(mm == 31),
                                    )
                                mm += 1
                        for k in range(2):
                            st = pk * 2 + k
                            nc.scalar.copy(ob[:, st, :], pso[k][:])
                else:
                    # last chunk: 4 banks (2 op + 2 borrowed pv), single pass
                    # over all 4 seq-tiles in part-major order to minimize the
                    # tail after the final gather.
                    pso = [
                        (op_ps if k < 2 else pv_ps).tile(
                            [P, CH], F32, tag="pv" if k >= 2 else "op",
                            name=f"pso{I}_{k}",
                        )
                        for k in range(ST)
                    ]
                    mm = 0
                    for part in range(NAG):
                        for at, jj, g in mm_rows(part):
                            for st in range(ST):
                                nc.tensor.matmul(
                                    pso[st][:],
                                    at[:, jj, st * P : (st + 1) * P],
                                    wo_sb[0][:, g, :],
                                    start=(mm == 0),
                                    stop=(mm == 31),
                                )
                            mm += 1
                    for st in range(ST):
                        nc.scalar.copy(ob[:, st, :], pso[st][:])
                nc.sync.dma_start(
                    out=out_ext[I * CH : (I + 1) * CH, :].rearrange(
                        "(po pi) e -> pi po e", pi=P
                    ),
                    in_=ob[:],
                )

            # ---- emission schedule ----
            # warmup collective: absorb the first-collective rendezvous cost
            # while the prologue DMAs run. Input is a copy of cossin rows.
            warm_in = ccin_pool.tile([P, 16], BF16, tag="wi", name="warm_in")
            nc.scalar.dma_start(out=warm_in[:], in_=cossin_ext[:, 0, 0:16])
            warm_out = gath_pool.tile(
                [P * N_CORES, 16], BF16, addr_space="Shared", tag="wo", name="warm_out"
            )
            nc.gpsimd.collective_compute(
                "AllGather",
                mybir.AluOpType.bypass,
                replica_groups=[list(range(N_CORES))],
                ins=[warm_in[:].opt()],
                outs=[warm_out[:].opt()],
            )

            # prologue: interleave wqkv groups with x chunk-0 groups
            nc.scalar.dma_start(out=cossin_sb[:], in_=cossin_ext[:])
            w_eng = [nc.gpsimd, nc.scalar, nc.sync, nc.gpsimd]
            x_eng = [nc.sync, nc.gpsimd, nc.scalar, nc.sync]
            for g in range(4):
                wb = wqkv_pool.tile([P, 8, QKV], BF16, tag="wqkv", name=f"wqkv{g}")
                nsub = 4
                for j in range(nsub):
                    w = 8 // nsub
                    w_eng[g].dma_start(
                        out=wb[:, j * w : (j + 1) * w, :],
                        in_=wqkv_ext[
                            (g * 8 + j * w) * P : (g * 8 + (j + 1) * w) * P, :
                        ].rearrange("(po pi) c -> pi po c", pi=P),
                    )
                wqkv_t[g] = wb
                xb = x_pool.tile([P, 8, CH], BF16, tag="xbf", name=f"xbf0_{g}")
                for j in range(nsub):
                    w = 8 // nsub
                    x_eng[g].dma_start(
                        out=xb[:, j * w : (j + 1) * w, :],
                        in_=xT_ext[
                            (g * 8 + j * w) * P : (g * 8 + (j + 1) * w) * P, 0:CH
                        ].rearrange("(po pi) s -> pi po s", pi=P),
                    )
                x_tiles[(0, g)] = xb
                if g == 3:
                    nc.scalar.dma_start(
                        out=mask_all[:], in_=mask_ext[:].rearrange("r p c -> p r c")
                    )
            emit_qkv0()
            emit_x_chunk(1)
            # wo resident load (needed first by outproj(0), ~2 chunks in)
            wo_t = wo_pool.tile([P, DT, DC], BF16, tag="wo", name="wo_sb")
            for g in range(4):
                nc.gpsimd.dma_start(
                    out=wo_t[:, g * 8 : (g + 1) * 8, :],
                    in_=wo_ext[g * 8 * P : (g + 1) * 8 * P, :].rearrange(
                        "(po pi) e -> pi po e", pi=P
                    ),
                )
            wo_sb[0] = wo_t
            emit_attn(0)

            emit_qkv(1)
            emit_x_chunk(2)
            emit_attn(1)
            emit_outproj(0)

            emit_qkv(2)
            emit_x_chunk(3)
            emit_attn(2)
            emit_outproj(1)

            emit_qkv(3)
            emit_attn(3)
            emit_outproj(2)
            emit_outproj(3)

    nc.compile()
    return nc


def _prep_inputs(x, freqs_cos, freqs_sin, wq, wk, wv, wo):
    """Shard + lay out the full inputs for the 8 cores."""
    xT = np.ascontiguousarray(x.reshape(S, D).T).astype(np.float32)
    cosT = np.ascontiguousarray(freqs_cos.T).astype(np.float32)
    sinT = np.ascontiguousarray(freqs_sin.T).astype(np.float32)

    perm = np.concatenate([np.arange(0, HD, 2), np.arange(1, HD, 2)])

    import ml_dtypes

    bf = ml_dtypes.bfloat16
    jj = np.arange(P)[:, None]
    ii = np.arange(P)[None, :]
    masks = np.zeros((2, P, P), bf)
    masks[0] = (ii <= jj).astype(bf)  # window-edge triangle
    masks[1] = (ii >= jj).astype(bf)  # causal triangle
    xT_bf = xT.astype(bf)
    cc2 = np.concatenate([cosT, cosT], axis=0)  # [128, S]
    ss2 = np.concatenate([sinT, sinT], axis=0)
    cossinT_bf = np.stack([cc2, ss2], axis=1).astype(bf)  # [128, 2, S]
    in_maps = []
    for c in range(N_CORES):
        q_cols = np.concatenate([(QH * c + h) * HD + perm for h in range(QH)])
        k_cols = c * HD + perm
        wqkv = np.concatenate(
            [wq[:, q_cols], wk[:, k_cols], wv[:, c * HD : (c + 1) * HD]], axis=1
        )
        in_maps.append(
            {
                "xT": xT_bf,
                "wqkv": np.ascontiguousarray(wqkv).astype(bf),
                "wo": np.ascontiguousarray(wo[:, c * DC : (c + 1) * DC]).astype(bf),
                "cossinT": cossinT_bf,
                "masks": masks,
            }
        )
    return in_maps


def kernel(x, freqs_cos, freqs_sin, wq, wk, wv, wo, _trace=False, _result_box=None):
    global _BUILT
    x = np.asarray(x, dtype=np.float32)
    if _BUILT is None:
        _BUILT = _build()
    nc = _BUILT
    in_maps = _prep_inputs(
        x,
        np.asarray(freqs_cos, np.float32),
        np.asarray(freqs_sin, np.float32),
        np.asarray(wq, np.float32),
        np.asarray(wk, np.float32),
        np.asarray(wv, np.float32),
        np.asarray(wo, np.float32),
    )
    res = run_bass_kernel_spmd(nc, in_maps, core_ids=list(range(N_CORES)), trace=_trace)
    if _result_box is not None:
        _result_box.append(res)
    out = np.concatenate(
        [np.asarray(res.results[c]["out"], dtype=np.float32) for c in range(N_CORES)],
        axis=1,
    )
    return out.reshape(B, S, D).astype(np.float32)
